# revision 6
# baseline (speedup 1.0000x reference)
"""Trainium2 Bass kernel for nn_MultiHeadAttention_37349035606665.

Multi-head attention with 2D RoPE:
  x [16, 1024, 1024] -> qkv -> rope(q,k) -> softmax(q k^T / 8) v -> proj

Sharding: data-parallel over batch. 8 cores x 2 batches each. Full inputs in,
full output out; host slices/concats.

Per-core dataflow (per batch):
  - host passes xT (x transposed to [c, n]) so contraction dim c lands on SBUF
    partitions for both qkv matmul operands.
  - qkT computed in [c_out, n] layout (heads on partitions) with the q/k head
    dims permuted pair-adjacent (host permutes W rows) so RoPE's rotate-half
    becomes a partition pair-swap (stream_shuffle) + 2 muls + add.
  - S_T[m, n] = k_rope^T q_rope per head, two heads row-packed on the PE
    (K=64 each, rows 0-63 / 64-127). exp(S/8) on ScalarE -> P_T (bf16).
  - O_T[d, n] = V^T P_T col-packed (two heads at array cols 0-63/64-127),
    plus ones-matmuls producing row-replicated softmax denominators.
  - normalize: reciprocal_approx + tensor_tensor mult (one op per pair).
  - proj: y[n, c_out] = O_T^T @ WpT + bias via K=1 ones matmul.

Matmul dtypes: float32r (reduced-precision single-pass fp32, ~1.6e-4) on the
q/k/S path; bf16 on the P*V and proj path (needed for PE col-tiling).
"""
import sys
sys.path.insert(0, "/opt/trn_rl_repo")
import numpy as np

DIM = 1024
NUM_HEADS = 16
HEAD_DIM = 64
GRID = 32
N = GRID * GRID          # 1024 tokens
THETA = 10000.0
B = 16
N_CORES = 8
B_LOC = B // N_CORES     # 2 batches per core
NPAIRS = NUM_HEADS // 2  # 8 head pairs
KT = DIM // 128          # 8 k-tiles
NT = N // 128            # 8 n-tiles

_cache = {}


def _build_nc():
    import concourse.bacc as bacc
    import concourse.mybir as mybir
    from concourse.tile import TileContext

    f32 = mybir.dt.float32
    f32r = mybir.dt.float32r
    bf16 = mybir.dt.bfloat16
    AF = mybir.ActivationFunctionType

    nc = bacc.Bacc(None, target_bir_lowering=False)

    xT_d = nc.declare_dram_parameter("xT", [B_LOC, DIM, N], f32, isOutput=False)
    wqk_d = nc.declare_dram_parameter("wqk", [DIM, 2 * DIM], f32, isOutput=False)
    qkb_d = nc.declare_dram_parameter("qkb", [2 * DIM, 1], f32, isOutput=False)
    wv_d = nc.declare_dram_parameter("wv", [DIM, DIM], f32, isOutput=False)
    wp_d = nc.declare_dram_parameter("wp", [DIM, DIM], f32, isOutput=False)
    bpe_d = nc.declare_dram_parameter("bpe", [1, DIM], f32, isOutput=False)
    cos_d = nc.declare_dram_parameter("cosT", [128, N], f32, isOutput=False)
    sin_d = nc.declare_dram_parameter("sinT", [128, N], f32, isOutput=False)
    y_d = nc.declare_dram_parameter("y", [B_LOC, N, DIM], f32, isOutput=True)

    with TileContext(nc) as tc:
        with tc.tile_pool(name="const", bufs=1) as cpool, \
             tc.tile_pool(name="wres", bufs=1) as wpool, \
             tc.tile_pool(name="xt", bufs=8) as xpool, \
             tc.tile_pool(name="wstream", bufs=2) as wspool, \
             tc.tile_pool(name="qk", bufs=6) as qkpool, \
             tc.tile_pool(name="rtmp", bufs=3) as rpool, \
             tc.tile_pool(name="vbf", bufs=8) as vpool, \
             tc.tile_pool(name="pt", bufs=16) as ptpool, \
             tc.tile_pool(name="rb", bufs=1) as rbpool, \
             tc.tile_pool(name="onrm", bufs=8) as opool, \
             tc.tile_pool(name="yst", bufs=1) as ypool, \
             tc.tile_pool(name="ps", bufs=4, space="PSUM") as psp:

            # ---- constants ----
            cos_sb = cpool.tile([128, N], f32)
            nc.sync.dma_start(out=cos_sb, in_=cos_d[:, :])
            sin_sb = cpool.tile([128, N], f32)
            nc.sync.dma_start(out=sin_sb, in_=sin_d[:, :])
            bpe_bf = cpool.tile([1, DIM], bf16)
            nc.gpsimd.dma_start(out=bpe_bf, in_=bpe_d[:, :])
            ones_f = cpool.tile([128, 64], f32)
            nc.vector.memset(ones_f[:, :], 1.0)
            ones64_bf = cpool.tile([128, 64], bf16)
            nc.vector.tensor_copy(ones64_bf[:, :], ones_f[:, :])
            ones1_f = cpool.tile([1, 128], f32)
            nc.vector.memset(ones1_f[:, :], 1.0)
            ones1_bf = cpool.tile([1, 128], bf16)
            nc.vector.tensor_copy(ones1_bf[:, :], ones1_f[:, :])
            qkb_sb = []
            for t in range(16):
                bt = cpool.tile([128, 1], f32, name=f"qkb{t}")
                nc.sync.dma_start(out=bt, in_=qkb_d[t * 128:(t + 1) * 128, :])
                qkb_sb.append(bt)
            # resident weights: wv (f32r), all 8 k-tiles
            wv_sb = []
            for kt in range(KT):
                wt = wpool.tile([128, DIM], f32r, name=f"wv{kt}")
                nc.gpsimd.dma_start(out=wt, in_=wv_d[kt * 128:(kt + 1) * 128, :])
                wv_sb.append(wt)
            wp_bf = []
            for kt in range(KT):
                wt = wpool.tile([128, DIM], bf16, name=f"wp{kt}")
                nc.gpsimd.dma_start(out=wt, in_=wp_d[kt * 128:(kt + 1) * 128, :])
                wp_bf.append(wt)

            swap_mask = [j ^ 1 for j in range(32)]

            for b in range(B_LOC):
                # ---- load xT (f32r cast-DMA) ----
                xt = []
                for kt in range(KT):
                    t = xpool.tile([128, N], mybir.dt.float32r, name=f"xt{kt}", tag="xt")
                    nc.gpsimd.dma_start(out=t, in_=xT_d[b, kt * 128:(kt + 1) * 128, :])
                    xt.append(t)

                # ---- v = xT^T @ WvT   [n, c] natural, stored bf16 ----
                v_bf = []
                for mt in range(NT):
                    ps_v = psp.tile([128, N], f32, name=f"psv{mt}", tag="ps")
                    for ch in range(2):
                        for kt in range(KT):
                            nc.tensor.matmul(
                                ps_v[:, ch * 512:(ch + 1) * 512],
                                xt[kt][:, mt * 128:(mt + 1) * 128],
                                wv_sb[kt][:, ch * 512:(ch + 1) * 512],
                                start=(kt == 0), stop=(kt == KT - 1))
                    vt = vpool.tile([128, DIM], bf16, name=f"v{mt}", tag="vbf")
                    nc.vector.tensor_copy(vt[:, :], ps_v[:, :])
                    v_bf.append(vt)

                o_norm = []
                for p in range(NPAIRS):
                    # ---- qk projection for this pair (q tile t=p, k tile t=8+p) ----
                    # columns [p*256, p*256+256) hold [q-pair | k-pair] (host interleaved)
                    rot = []
                    wqk_tiles = []
                    for kt in range(KT):
                        wt = wspool.tile([128, 256], f32r, name=f"wqk{b}_{p}_{kt}", tag="wqk", bufs=8)
                        nc.gpsimd.dma_start(out=wt, in_=wqk_d[kt * 128:(kt + 1) * 128,
                                                             p * 256:(p + 1) * 256])
                        wqk_tiles.append(wt)

                    for qk_i in range(2):
                        ps_qk = psp.tile([128, N], f32, name=f"psqk{b}_{p}_{qk_i}", tag="ps")
                        for ch in range(2):
                            for kt in range(KT):
                                nc.tensor.matmul(
                                    ps_qk[:, ch * 512:(ch + 1) * 512],
                                    wqk_tiles[kt][:, qk_i * 128:(qk_i + 1) * 128],
                                    xt[kt][:, ch * 512:(ch + 1) * 512],
                                    start=(kt == 0), stop=(kt == KT - 1))
                        # bias add (per-partition) psum -> sbuf fp32
                        t_idx = p if qk_i == 0 else 8 + p
                        qk_sb = qkpool.tile([128, N], f32, name=f"qks{b}_{p}_{qk_i}", tag="qk")
                        nc.scalar.activation(qk_sb[:, :], ps_qk[:, :], AF.Identity,
                                             bias=qkb_sb[t_idx][:, :], scale=1.0)
                        # rope: rot = qk*cos + swap(qk)*sin_signed   -> f32r
                        qsw = rpool.tile([128, N], f32, name=f"qsw{b}_{p}_{qk_i}", tag="rtmp")
                        nc.vector.stream_shuffle(qsw[:, :], qk_sb[:, :], swap_mask)
                        m1 = rpool.tile([128, N], f32, name=f"m1{b}_{p}_{qk_i}", tag="rtmp")
                        nc.vector.tensor_tensor(out=m1[:, :], in0=qk_sb[:, :],
                                                in1=cos_sb[:, :], op=mybir.AluOpType.mult)
                        m2 = rpool.tile([128, N], f32, name=f"m2{b}_{p}_{qk_i}", tag="rtmp")
                        nc.gpsimd.tensor_tensor(out=m2[:, :], in0=qsw[:, :],
                                                in1=sin_sb[:, :], op=mybir.AluOpType.mult)
                        rt = qkpool.tile([128, N], f32r, name=f"rot{b}_{p}_{qk_i}", tag="qk")
                        nc.vector.tensor_tensor(out=rt[:, :], in0=m1[:, :], in1=m2[:, :],
                                                op=mybir.AluOpType.add)
                        rot.append(rt)
                    rot_q, rot_k = rot

                    # ---- S_T + exp -> P_T (bf16), row-packed pair ----
                    pt_a, pt_b = [], []
                    for mt in range(NT):
                        ps_a = psp.tile([128, N], f32, name=f"psa{b}_{p}_{mt}", tag="ps")
                        ps_b = psp.tile([128, N], f32, name=f"psb{b}_{p}_{mt}", tag="ps")
                        for ch in range(2):
                            nc.tensor.matmul(
                                ps_a[:, ch * 512:(ch + 1) * 512],
                                rot_k[0:64, mt * 128:(mt + 1) * 128],
                                rot_q[0:64, ch * 512:(ch + 1) * 512],
                                start=True, stop=True, tile_position=(0, 0))
                            nc.tensor.matmul(
                                ps_b[:, ch * 512:(ch + 1) * 512],
                                rot_k[64:128, mt * 128:(mt + 1) * 128],
                                rot_q[64:128, ch * 512:(ch + 1) * 512],
                                start=True, stop=True, tile_position=(64, 0))
                        ta = ptpool.tile([128, N], bf16, name=f"pta{b}_{p}_{mt}", tag="pt")
                        nc.scalar.activation(ta[:, :], ps_a[:, :], AF.Exp, scale=0.125)
                        pt_a.append(ta)
                        tb = ptpool.tile([128, N], bf16, name=f"ptb{b}_{p}_{mt}", tag="pt")
                        nc.scalar.activation(tb[:, :], ps_b[:, :], AF.Exp, scale=0.125)
                        pt_b.append(tb)

                    # ---- PV + s, col-packed ----
                    ps_o = psp.tile([128, N], f32, name=f"pso{b}_{p}", tag="ps")
                    ps_s = psp.tile([128, N], f32, name=f"pss{b}_{p}", tag="ps")
                    for ch in range(2):
                        sl = slice(ch * 512, (ch + 1) * 512)
                        for mt in range(NT):
                            st = (mt == 0)
                            sp = (mt == NT - 1)
                            nc.tensor.matmul(ps_o[0:64, sl],
                                             v_bf[mt][:, (2 * p) * 64:(2 * p + 1) * 64],
                                             pt_a[mt][:, sl], start=st, stop=sp,
                                             tile_position=(0, 0))
                            nc.tensor.matmul(ps_o[64:128, sl],
                                             v_bf[mt][:, (2 * p + 1) * 64:(2 * p + 2) * 64],
                                             pt_b[mt][:, sl], start=st, stop=sp,
                                             tile_position=(0, 64))
                            nc.tensor.matmul(ps_s[0:64, sl], ones64_bf[:, :],
                                             pt_a[mt][:, sl], start=st, stop=sp,
                                             tile_position=(0, 0))
                            nc.tensor.matmul(ps_s[64:128, sl], ones64_bf[:, :],
                                             pt_b[mt][:, sl], start=st, stop=sp,
                                             tile_position=(0, 64))
                    rb = rbpool.tile([128, N], f32, name=f"rb{b}_{p}", tag="rb")
                    nc.vector.reciprocal_approx_fast(out=rb[:, :], in_=ps_s[:, :])
                    ot = opool.tile([128, N], bf16, name=f"on{b}_{p}", tag="onrm")
                    nc.vector.tensor_tensor(out=ot[:, :], in0=ps_o[:, :], in1=rb[:, :],
                                            op=mybir.AluOpType.mult)
                    o_norm.append(ot)

                # ---- proj: y[n, c_out] ----
                for nt in range(NT):
                    ps_y = psp.tile([128, N], f32, name=f"psy{b}_{nt}", tag="ps")
                    for ch in range(2):
                        sl = slice(ch * 512, (ch + 1) * 512)
                        for p in range(NPAIRS):
                            nc.tensor.matmul(ps_y[:, sl],
                                             o_norm[p][:, nt * 128:(nt + 1) * 128],
                                             wp_bf[p][:, sl], start=(p == 0), stop=False)
                        nc.tensor.matmul(ps_y[:, sl], ones1_bf[:, :], bpe_bf[:, sl],
                                         start=False, stop=True)
                    yt = ypool.tile([128, DIM], f32, name=f"y{b}_{nt}", tag="yst")
                    nc.vector.tensor_copy(yt[:, :], ps_y[:, :])
                    nc.sync.dma_start(out=y_d[b, nt * 128:(nt + 1) * 128, :], in_=yt[:, :])

    nc.finalize()
    return nc


def _host_prep(x, qkv_w, qkv_b, proj_w, proj_b):
    """Build per-core input maps (host-side layout prep + sharding)."""
    # pair-adjacent permutation of q/k head dims: within each head,
    # order [0, 32, 1, 33, ..., 31, 63]
    half = HEAD_DIM // 2
    perm_head = np.empty(HEAD_DIM, np.int64)
    perm_head[0::2] = np.arange(half)
    perm_head[1::2] = np.arange(half) + half
    perm = np.concatenate([h * HEAD_DIM + perm_head for h in range(NUM_HEADS)])

    wq = qkv_w[0:DIM][perm]            # [1024, 1024] rows permuted
    wk = qkv_w[DIM:2 * DIM][perm]
    bq = qkv_b[0:DIM][perm]
    bk = qkv_b[DIM:2 * DIM][perm]
    wv = qkv_w[2 * DIM:3 * DIM]
    bv = qkv_b[2 * DIM:3 * DIM]

    # wqk layout: [c_in, pair-blocks of 256 = [q-pair 128 | k-pair 128]]
    wqk = np.empty((DIM, 2 * DIM), np.float32)
    qkb = np.empty((2 * DIM, 1), np.float32)
    for p in range(NPAIRS):
        wqk[:, p * 256:p * 256 + 128] = wq[p * 128:(p + 1) * 128].T
        wqk[:, p * 256 + 128:(p + 1) * 256] = wk[p * 128:(p + 1) * 128].T
    # bias tiles: t in 0..7 -> q pair t ; t in 8..15 -> k pair t-8
    for p in range(NPAIRS):
        qkb[p * 128:(p + 1) * 128, 0] = bq[p * 128:(p + 1) * 128]
        qkb[1024 + p * 128:1024 + (p + 1) * 128, 0] = bk[p * 128:(p + 1) * 128]

    wvT = np.ascontiguousarray(wv.T)                       # [c_in, c_out_v]
    wpT = np.ascontiguousarray(proj_w.T)                   # [c, c_out]
    bpe = (proj_b + bv @ proj_w.T).astype(np.float32)[None, :]

    # rope tables in pair-adjacent layout [128 rows = 2 heads x 64, N]
    freqs = 1.0 / (THETA ** (np.arange(0, HEAD_DIM, 2, dtype=np.float32) / HEAD_DIM))
    f = freqs[: half // 2]                                 # [16]
    y_, x_ = np.meshgrid(np.arange(GRID), np.arange(GRID), indexing="ij")
    pos_y = y_.reshape(-1).astype(np.float32)
    pos_x = x_.reshape(-1).astype(np.float32)
    fr = np.concatenate([pos_y[:, None] * f[None, :],
                         pos_x[:, None] * f[None, :]], axis=-1)  # [N, 32]
    cos = np.cos(fr)                                       # [N, 32]
    sin = np.sin(fr)
    cosT = np.empty((128, N), np.float32)
    sinT = np.empty((128, N), np.float32)
    for pi in range(64):
        i = pi // 2
        parity = pi % 2
        for hh in range(2):
            row = hh * 64 + pi
            cosT[row] = cos[:, i]
            sinT[row] = (sin[:, i] if parity == 1 else -sin[:, i])

    xT = np.ascontiguousarray(x.transpose(0, 2, 1))        # [B, c, n]

    in_maps = []
    for core in range(N_CORES):
        in_maps.append({
            "xT": xT[core * B_LOC:(core + 1) * B_LOC],
            "wqk": wqk, "qkb": qkb, "wv": wvT, "wp": wpT,
            "bpe": bpe, "cosT": cosT, "sinT": sinT,
        })
    return in_maps


def kernel(x, qkv_w, qkv_b, proj_w, proj_b):
    from concourse.bass_utils import run_bass_kernel_spmd

    x = np.asarray(x, np.float32)
    qkv_w = np.asarray(qkv_w, np.float32)
    qkv_b = np.asarray(qkv_b, np.float32)
    proj_w = np.asarray(proj_w, np.float32)
    proj_b = np.asarray(proj_b, np.float32)

    if "nc" not in _cache:
        _cache["nc"] = _build_nc()
    nc = _cache["nc"]

    in_maps = _host_prep(x, qkv_w, qkv_b, proj_w, proj_b)
    res = run_bass_kernel_spmd(nc, in_maps, core_ids=list(range(N_CORES)))
    _cache["last_result"] = res
    out = np.concatenate([res.results[c]["y"] for c in range(N_CORES)], axis=0)
    return out
